# revision 1
# baseline (speedup 1.0000x reference)
"""BinaryMLP (dense_mlp) Trainium2 kernel — 8-core data-parallel sync-BN.

Strategy:
  - Shard batch (4096) across 8 NeuronCores (512 rows each); replicate weights.
  - Activations live in SBUF transposed: [features -> partitions, batch -> free].
    BatchNorm batch stats are then free-axis reductions (VectorE / ACT accum).
  - Matmuls: lhsT = W.T tile (stationary), rhs = xT tile (moving), bf16 in,
    fp32 PSUM accumulation. Weight sign() / transposes / casts done host-side.
  - Sync-BN: per-feature sum / sumsq reduced locally, then one 32KB fp32
    AllReduce per BN layer across the 8 cores.
  - Final Linear flips layout back to [batch -> partitions, classes -> free] by
    using the h3 activation tiles as the stationary operand; log_softmax is a
    free-axis max/exp-accum/ln chain; b3 is folded in via an extra
    ones-row x b3-row contraction tile.
"""

import os
import sys

for _p in ("/opt/trn_rl_repo",):
    if _p not in sys.path and os.path.isdir(_p):
        sys.path.insert(0, _p)

import numpy as np
import ml_dtypes

import concourse.bass as bass
import concourse.mybir as mybir
import concourse.tile as tile
from concourse import bacc
from concourse.bass_utils import run_bass_kernel_spmd

AF = mybir.ActivationFunctionType
ALU = mybir.AluOpType
F32 = mybir.dt.float32
BF16 = mybir.dt.bfloat16
AX = mybir.AxisListType

NP_BF16 = ml_dtypes.bfloat16

P = 128
N_CORES = 8
B_TOTAL = 4096
D_IN = 4096
H1, H2, H3 = 4096, 4096, 2048
C = 1000
BN_EPS = 1e-5

KT0, MT0 = D_IN // P, H1 // P  # 32, 32
KT1, MT1 = H1 // P, H2 // P  # 32, 32
KT2, MT2 = H2 // P, H3 // P  # 32, 16
KT3 = H3 // P  # 16 (+1 aug tile for the bias)
MG = 4  # out-feature tiles per PSUM group (4 banks; 2 groups in flight)
KPAIR = 4  # k-tiles per weight-slab DMA (512KB slabs amortize DMA fixed cost)


def build(b_shard: int, n_cores: int):
    """Build + compile the SPMD program for a per-core batch shard of b_shard."""
    assert b_shard % P == 0
    nb = b_shard // P  # batch tiles for the final layer
    n_batch_global = b_shard * n_cores
    rg = [list(range(n_cores))]

    nc = bacc.Bacc(
        "TRN2", target_bir_lowering=False, debug=False, num_devices=n_cores
    )

    xT = nc.dram_tensor("xT", [D_IN, b_shard], BF16, kind="ExternalInput").ap()
    w0t = nc.dram_tensor("w0t", [D_IN, H1], BF16, kind="ExternalInput").ap()
    w1t = nc.dram_tensor("w1t", [H1, H2], BF16, kind="ExternalInput").ap()
    w2t = nc.dram_tensor("w2t", [H2, H3], BF16, kind="ExternalInput").ap()
    # W3.T augmented with a b3 row (row H3) + zero padding to a full k-tile.
    w3t = nc.dram_tensor("w3t", [(KT3 + 1) * P, C], BF16, kind="ExternalInput").ap()
    g0p = nc.dram_tensor("g0p", [P, MT0], F32, kind="ExternalInput").ap()
    b0p = nc.dram_tensor("b0p", [P, MT0], F32, kind="ExternalInput").ap()
    g1p = nc.dram_tensor("g1p", [P, MT1], F32, kind="ExternalInput").ap()
    b1p = nc.dram_tensor("b1p", [P, MT1], F32, kind="ExternalInput").ap()
    g2p = nc.dram_tensor("g2p", [P, MT2], F32, kind="ExternalInput").ap()
    b2p = nc.dram_tensor("b2p", [P, MT2], F32, kind="ExternalInput").ap()
    out = nc.dram_tensor("out", [b_shard, C], F32, kind="ExternalOutput").ap()

    with tile.TileContext(nc) as tc:
        with (
            tc.tile_pool(name="big", bufs=1) as big,
            tc.tile_pool(name="wpool", bufs=5) as wpool,
            tc.tile_pool(name="psum", bufs=8, space="PSUM") as psum,
            tc.tile_pool(name="scratch", bufs=4) as scratch,
            tc.tile_pool(name="bn", bufs=8) as bnp,
            tc.tile_pool(name="small", bufs=24) as small,
            tc.tile_pool(name="dram", bufs=1, space="DRAM") as dram,
        ):
            # ---- persistent activation buffers -------------------------------
            xT_sb = big.tile([P, KT0, b_shard], BF16, name="xT_sb", tag="xT_sb")
            h1_sb = big.tile([P, MT0, b_shard], BF16, name="h1_sb", tag="h1_sb")
            h2_sb = big.tile([P, MT1, b_shard], BF16, name="h2_sb", tag="h2_sb")
            h3_sb = big.tile([P, MT2, b_shard], BF16, name="h3_sb", tag="h3_sb")
            ones_t = big.tile([P, b_shard], BF16, name="ones_t", tag="ones_t")

            nc.gpsimd.memset(ones_t[:], 0.0)
            nc.gpsimd.memset(ones_t[:1, :], 1.0)
            # xT loads in 512KB chunks, emitted just-in-time on the sync
            # queue interleaved with the weight-slab stream (one-chunk
            # lookahead) so the first matmuls start within a few us.
            xT_r = xT.rearrange("(ko p) b -> p ko b", p=P)
            XCH = 4  # k-tiles per xT chunk
            n_xch = KT0 // XCH
            xch_emitted = [False] * n_xch

            def xT_loader(kp):
                # sync queue, just-in-time before the slab that needs it
                # (HWDGE ramps faster than SWDGE for these strided chunks)
                want = min((kp + KPAIR - 1) // XCH, n_xch - 1)
                for c in range(want + 1):
                    if not xch_emitted[c]:
                        xch_emitted[c] = True
                        nc.sync.dma_start(
                            xT_sb[:, c * XCH : (c + 1) * XCH, :],
                            xT_r[:, c * XCH : (c + 1) * XCH, :],
                        )

            # BN gamma/beta (host packed to [P, MT])
            gb = {}

            def load_gb(specs):
                for nm, ap_, mt in specs:
                    t = big.tile([P, mt], F32, name=f"{nm}_sb", tag=f"{nm}_sb")
                    nc.gpsimd.dma_start(t[:], ap_)
                    gb[nm] = t

            # only layer 0's params up front; the rest load after layer 0's
            # emission so the startup gpsimd queue stays clear for xT chunks
            load_gb((("g0", g0p, MT0), ("b0", b0p, MT0)))

            # warm the ACT Exp/Ln LUTs now (ACT is idle) so the softmax tail
            # doesn't pay the 1.28us table load on its critical path
            warm = small.tile([P, 1], F32, name="warm", tag="sm")
            nc.gpsimd.memset(warm[:], 1.0)
            nc.scalar.activation(warm[:], warm[:], AF.Exp)
            nc.scalar.activation(warm[:], warm[:], AF.Ln)

            def mlp_layer(
                lidx, in_sb, kt, mt, w_dram, g_sb, b_sb, out_sb,
                a_groups=None, cover_k=None, in_loader=None,
            ):
                """out_sb <- relu(bn(in_sb.T @ W.T)), transposed layout.

                Pipelined sync-BN: per-feature stats are all-reduced in two
                chunks (chunk-a = first a_groups PSUM groups); AR-a overlaps
                this layer's later matmuls and AR-b overlaps the NEXT layer's
                matmuls, whose k-loops are split at cover_k so input tiles
                normalized by the previous chunk-a are consumed first.
                """
                ngroups = mt // MG
                half_g = a_groups if a_groups is not None else ngroups // 2
                a_m = half_g * MG  # chunk-a feature tiles
                b_m = mt - a_m
                kh = cover_k if cover_k is not None else kt // 2
                inv_n = 1.0 / float(n_batch_global)
                chunk_m = [a_m, b_m]

                stats = [
                    big.tile(
                        [P, 2 * chunk_m[c]], F32, name=f"stats{lidx}_{c}",
                        tag=f"stats{lidx}_{c}",
                    )
                    for c in range(2)
                ]
                gstats = [
                    big.tile(
                        [P, 2 * chunk_m[c]], F32, name=f"gstats{lidx}_{c}",
                        tag=f"gstats{lidx}_{c}",
                    )
                    for c in range(2)
                ]
                arin = [
                    dram.tile(
                        [P, 2 * chunk_m[c]], F32, name=f"arin{lidx}_{c}",
                        tag=f"arin{lidx}_{c}",
                    )
                    for c in range(2)
                ]
                arout = [
                    dram.tile(
                        [P, 2 * chunk_m[c]], F32, name=f"arout{lidx}_{c}",
                        tag=f"arout{lidx}_{c}",
                    )
                    for c in range(2)
                ]
                s_sb = [
                    big.tile(
                        [P, chunk_m[c]], F32, name=f"s{lidx}_{c}", tag=f"s{lidx}_{c}"
                    )
                    for c in range(2)
                ]
                t_sb = [
                    big.tile(
                        [P, chunk_m[c]], F32, name=f"t{lidx}_{c}", tag=f"t{lidx}_{c}"
                    )
                    for c in range(2)
                ]

                ps_tiles = {}

                def emit_mms(g, k_lo, k_hi):
                    if g not in ps_tiles:
                        ps_tiles[g] = [
                            psum.tile(
                                [P, b_shard], F32, name=f"ps{lidx}_{g}_{j}", tag="ps"
                            )
                            for j in range(MG)
                        ]
                    ps = ps_tiles[g]
                    for kp in range(k_lo, k_hi, KPAIR):
                        if in_loader is not None:
                            in_loader(kp)
                        slab = wpool.tile(
                            [P, KPAIR, MG * P], BF16,
                            name=f"w{lidx}_{g}_{kp}", tag="wslab",
                        )
                        nc.sync.dma_start(
                            slab[:],
                            w_dram[
                                kp * P : (kp + KPAIR) * P,
                                g * MG * P : (g + 1) * MG * P,
                            ].rearrange("(kk p) c -> p kk c", p=P),
                        )
                        for kk in range(KPAIR):
                            k = kp + kk
                            for j in range(MG):
                                nc.tensor.matmul(
                                    ps[j][:],
                                    slab[:, kk, j * P : (j + 1) * P],
                                    in_sb[:, k, :],
                                    start=(k == 0),
                                    stop=(k == kt - 1),
                                )

                def emit_stats(g):
                    c = 0 if g < half_g else 1
                    st = stats[c]
                    cm = chunk_m[c]
                    for j in range(MG):
                        m = g * MG + j
                        ml = m - c * a_m
                        nc.scalar.activation(out_sb[:, m, :], ps_tiles[g][j][:], AF.Copy)
                        nc.vector.tensor_reduce(
                            st[:, ml : ml + 1], ps_tiles[g][j][:], axis=AX.X, op=ALU.add
                        )
                        sq = scratch.tile(
                            [P, b_shard], F32, name=f"sq{lidx}_{m}", tag="sq"
                        )
                        # HW: only one PSUM read per DVE inst, so square on ACT
                        nc.scalar.activation(
                            sq[:],
                            ps_tiles[g][j][:],
                            AF.Square,
                            accum_out=st[:, cm + ml : cm + ml + 1],
                        )

                def emit_ar(c):
                    nc.gpsimd.dma_start(arin[c][:], stats[c][:])
                    nc.gpsimd.collective_compute(
                        "AllReduce",
                        ALU.add,
                        replica_groups=rg,
                        ins=[arin[c].opt()],
                        outs=[arout[c].opt()],
                    )
                    nc.gpsimd.dma_start(gstats[c][:], arout[c][:])

                def emit_apply(c):
                    # s = g * rsqrt(var+eps); t = beta - mean*s, then in-place
                    # relu(h*s + t) for this chunk's feature tiles.
                    gs = gstats[c]
                    cm = chunk_m[c]
                    m0 = c * a_m
                    mex = bnp.tile([P, 2 * cm], F32, name=f"mex{lidx}_{c}", tag="bn2")
                    m2 = bnp.tile([P, cm], F32, name=f"m2{lidx}_{c}", tag="bn")
                    var = bnp.tile([P, cm], F32, name=f"var{lidx}_{c}", tag="bn")
                    inv = bnp.tile([P, cm], F32, name=f"inv{lidx}_{c}", tag="bn")
                    rstd = bnp.tile([P, cm], F32, name=f"rstd{lidx}_{c}", tag="bn")
                    tmp = bnp.tile([P, cm], F32, name=f"tmp{lidx}_{c}", tag="bn")
                    nc.scalar.activation(mex[:], gs[:], AF.Copy, scale=inv_n)
                    mean = mex[:, :cm]
                    ex2 = mex[:, cm:]
                    nc.vector.tensor_mul(m2[:], mean[:], mean[:])
                    nc.vector.tensor_sub(var[:], ex2[:], m2[:])
                    nc.vector.tensor_scalar_add(var[:], var[:], BN_EPS)
                    nc.vector.reciprocal(inv[:], var[:])
                    nc.scalar.activation(rstd[:], inv[:], AF.Sqrt)
                    nc.vector.tensor_mul(
                        s_sb[c][:], rstd[:], g_sb[:, m0 : m0 + cm]
                    )
                    nc.vector.tensor_mul(tmp[:], mean[:], s_sb[c][:])
                    nc.vector.tensor_sub(
                        t_sb[c][:], b_sb[:, m0 : m0 + cm], tmp[:]
                    )
                    for ml in range(cm):
                        m = m0 + ml
                        nc.scalar.activation(
                            out_sb[:, m, :],
                            out_sb[:, m, :],
                            AF.Relu,
                            bias=t_sb[c][:, ml : ml + 1],
                            scale=s_sb[c][:, ml : ml + 1],
                        )

                # groups 0,1: k-loop split so the first half only needs the
                # previous layer's chunk-a (covers that layer's AR-b latency).
                # For the input layer, interleave in 4-k sub-blocks so the PE
                # can start as soon as the first 512KB input chunk lands.
                if in_loader is not None:
                    for k0 in range(0, kh, 4):
                        emit_mms(0, k0, k0 + 4)
                        emit_mms(1, k0, k0 + 4)
                else:
                    emit_mms(0, 0, kh)
                    emit_mms(1, 0, kh)
                emit_mms(0, kh, kt)
                emit_stats(0)
                emit_mms(1, kh, kt)
                emit_stats(1)
                for g in range(2, half_g):
                    emit_mms(g, 0, kt)
                    emit_stats(g)
                emit_ar(0)  # chunk-a stats AR overlaps chunk-b matmuls
                apply_a_at = half_g + 1 if (ngroups - half_g) > 2 else None
                apply_a_done = False
                for g in range(half_g, ngroups):
                    if apply_a_at is None and not apply_a_done:
                        # small chunk-b: wedge the chunk-a BN apply between
                        # this group's two half-k blocks so it runs as soon
                        # as AR-a lands, not after the whole layer
                        emit_mms(g, 0, kh)
                        emit_apply(0)
                        apply_a_done = True
                        emit_mms(g, kh, kt)
                        emit_stats(g)
                        continue
                    emit_mms(g, 0, kt)
                    emit_stats(g)
                    if apply_a_at is not None and g == apply_a_at:
                        emit_apply(0)
                        apply_a_done = True
                if not apply_a_done:
                    emit_apply(0)
                emit_ar(1)
                emit_apply(1)

            # chunk-a = 6 of 8 groups: the trailing AR covers ~50us of the
            # next layer's split matmuls (ARs vary 10-37us run to run)
            mlp_layer(
                0, xT_sb, KT0, MT0, w0t, gb["g0"], gb["b0"], h1_sb,
                a_groups=6, in_loader=xT_loader,
            )
            load_gb(
                (
                    ("g1", g1p, MT1),
                    ("b1", b1p, MT1),
                    ("g2", g2p, MT2),
                    ("b2", b2p, MT2),
                )
            )
            mlp_layer(
                1, h1_sb, KT1, MT1, w1t, gb["g1"], gb["b1"], h2_sb,
                a_groups=6, cover_k=24,
            )

            # preload ALL final-layer weight slabs now — the DMAs run during
            # layer 2's compute and layer 3 then never waits on weight loads
            # gpsimd (SWDGE) queue: runs during layer 2 without delaying the
            # sync-queue weight-slab stream
            w3_sb = big.tile([P, KT3 + 1, C], BF16, name="w3_sb", tag="w3_sb")
            nc.gpsimd.dma_start(
                w3_sb[:], w3t.rearrange("(ko p) c -> p ko c", p=P)
            )

            # asymmetric chunks (3+1 groups): the tiny chunk-b AR is covered
            # by layer 3's first 12 k-tiles
            mlp_layer(
                2, h2_sb, KT2, MT2, w2t, gb["g2"], gb["b2"], h3_sb,
                a_groups=3, cover_k=24,
            )

            # ---- final Linear + log_softmax ---------------------------------
            # lhsT = h3 tile slice (stationary), rhs = preloaded W3.T slab
            # (moving). Output flips to [batch -> partitions, classes -> free].
            # k-loop split: first 8 k-tiles (layer 2's chunk-a) for every
            # batch tile first, covering layer 2's second stats-AR.
            half = (C + 1) // 2  # 500
            ka = 12  # layer 2's chunk-a feature tiles
            ps3 = [
                [
                    psum.tile([P, 512], F32, name=f"ps3_{b}_{h}", tag="ps")
                    for h in range(2)
                ]
                for b in range(nb)
            ]

            def l3_mms(b, ks, stop_k, start_k=None):
                for k in ks:
                    lhsT = (
                        h3_sb[:, k, b * P : (b + 1) * P]
                        if k < KT3
                        else ones_t[:, b * P : (b + 1) * P]
                    )
                    for h in range(2):
                        nc.tensor.matmul(
                            ps3[b][h][:, : half],
                            lhsT,
                            w3_sb[:, k, h * half : (h + 1) * half],
                            start=(k == start_k),
                            stop=(k == stop_k),
                        )

            for b in range(nb):
                # bias (ones) k-tile first — it's ungated by layer 2's BN
                # applies, so it runs while layer 2's stats-ARs are in flight
                l3_mms(b, [KT3] + list(range(ka)), None, start_k=KT3)

            # log_softmax, stage-batched across batch tiles so the ACT LUT
            # (Exp / Ln) is loaded once per stage instead of per tile
            nmax = [None] * nb
            s0 = [None] * nb
            s1 = [None] * nb
            lse = [None] * nb
            shift = [None] * nb
            for b in range(nb):
                l3_mms(b, range(ka, KT3), KT3 - 1)
                p0 = ps3[b][0][:, :half]
                p1 = ps3[b][1][:, :half]
                m0 = small.tile([P, 1], F32, name=f"m0_{b}", tag="sm")
                m1 = small.tile([P, 1], F32, name=f"m1_{b}", tag="sm")
                nmax[b] = small.tile([P, 1], F32, name=f"nmax_{b}", tag="sm")
                nc.vector.tensor_reduce(m0[:], p0, axis=AX.X, op=ALU.max)
                nc.vector.tensor_reduce(m1[:], p1, axis=AX.X, op=ALU.max)
                nc.vector.tensor_max(m0[:], m0[:], m1[:])
                nc.vector.tensor_scalar_mul(nmax[b][:], m0[:], -1.0)
            for b in range(nb):
                s0[b] = small.tile([P, 1], F32, name=f"s0_{b}", tag="sm")
                s1[b] = small.tile([P, 1], F32, name=f"s1_{b}", tag="sm")
                e0 = scratch.tile([P, 512], F32, name=f"e0_{b}", tag="sq")
                e1 = scratch.tile([P, 512], F32, name=f"e1_{b}", tag="sq")
                nc.scalar.activation(
                    e0[:, :half], ps3[b][0][:, :half], AF.Exp,
                    bias=nmax[b][:], scale=1.0, accum_out=s0[b][:],
                )
                nc.scalar.activation(
                    e1[:, :half], ps3[b][1][:, :half], AF.Exp,
                    bias=nmax[b][:], scale=1.0, accum_out=s1[b][:],
                )
            for b in range(nb):
                ssum = small.tile([P, 1], F32, name=f"ssum_{b}", tag="sm")
                lse[b] = small.tile([P, 1], F32, name=f"lse_{b}", tag="sm")
                nc.vector.tensor_add(ssum[:], s0[b][:], s1[b][:])
                nc.scalar.activation(lse[b][:], ssum[:], AF.Ln)
            for b in range(nb):
                shift[b] = small.tile([P, 1], F32, name=f"shift_{b}", tag="sm")
                nc.vector.tensor_sub(shift[b][:], nmax[b][:], lse[b][:])
            for b in range(nb):
                # writeback split across ACT and DVE so the two halves of
                # each tile shift in parallel
                o0 = scratch.tile([P, 512], F32, name=f"o0_{b}", tag="sq")
                o1 = scratch.tile([P, 512], F32, name=f"o1_{b}", tag="sq")
                nc.scalar.activation(
                    o0[:, :half], ps3[b][0][:, :half], AF.Identity,
                    bias=shift[b][:], scale=1.0,
                )
                nc.vector.tensor_scalar_add(
                    o1[:, :half], ps3[b][1][:, :half], shift[b][:]
                )
                # halves on different DMA queues so the 8 stores drain in
                # parallel instead of serializing on sync
                nc.sync.dma_start(out[b * P : (b + 1) * P, :half], o0[:, :half])
                nc.gpsimd.dma_start(out[b * P : (b + 1) * P, half:C], o1[:, :half])

    nc.compile()
    return nc


def prep_inputs(inputs, b_shard: int, n_cores: int):
    """Host-side prep: shard x, transpose/cast weights, pack BN params."""
    x = np.ascontiguousarray(inputs["x"], dtype=np.float32)

    def bf(a):
        return np.ascontiguousarray(a).astype(NP_BF16)

    def sign_f32(w):
        return np.where(w >= 0, np.float32(1.0), np.float32(-1.0))

    w0t = bf(inputs["W0"].astype(np.float32).T)
    w1t = bf(sign_f32(np.asarray(inputs["Wb1"], dtype=np.float32)).T)
    w2t = bf(sign_f32(np.asarray(inputs["Wb2"], dtype=np.float32)).T)
    w3t_aug = np.zeros(((KT3 + 1) * P, C), dtype=np.float32)
    w3t_aug[:H3] = inputs["W3"].astype(np.float32).T
    w3t_aug[H3] = inputs["b3"].astype(np.float32)
    w3t_aug = bf(w3t_aug)

    def pack(v, mt):
        return np.ascontiguousarray(
            np.asarray(v, dtype=np.float32).reshape(mt, P).T
        )

    shared = {
        "w0t": w0t,
        "w1t": w1t,
        "w2t": w2t,
        "w3t": w3t_aug,
        "g0p": pack(inputs["g0"], MT0),
        "b0p": pack(inputs["beta0"], MT0),
        "g1p": pack(inputs["g1"], MT1),
        "b1p": pack(inputs["beta1"], MT1),
        "g2p": pack(inputs["g2"], MT2),
        "b2p": pack(inputs["beta2"], MT2),
    }
    in_maps = []
    for i in range(n_cores):
        xs = x[i * b_shard : (i + 1) * b_shard]  # [b_shard, D_IN]
        m = dict(shared)
        m["xT"] = bf(xs.T)  # [D_IN, b_shard]
        in_maps.append(m)
    return in_maps


_CACHE = {}


def _get_compiled(b_shard: int, n_cores: int):
    key = (b_shard, n_cores)
    if key not in _CACHE:
        _CACHE[key] = build(b_shard, n_cores)
    return _CACHE[key]


def kernel(**inputs) -> np.ndarray:
    b_shard = B_TOTAL // N_CORES
    nc = _get_compiled(b_shard, N_CORES)
    in_maps = prep_inputs(inputs, b_shard, N_CORES)
    last_err = None
    for _attempt in range(3):
        try:
            res = run_bass_kernel_spmd(nc, in_maps, core_ids=list(range(N_CORES)))
            break
        except Exception as e:  # transient NRT device flakes recover on retry
            last_err = e
    else:
        raise last_err
    out = np.concatenate([r["out"] for r in res.results], axis=0)
    return out.astype(np.float32)


if __name__ == "__main__":
    data = np.load("/tmp/ref_data.npz")
    inputs = {k: data[k] for k in data.files if k != "expected"}
    expected = data["expected"]
    actual = kernel(**inputs)
    err = np.abs(actual - expected)
    print("max abs err:", err.max())
    print("absmax-rel:", err.max() / np.abs(expected).max())



# revision 14
# speedup vs baseline: 1.2693x; 1.2693x over previous
"""BinaryMLP (dense_mlp) Trainium2 kernel — 8-core data-parallel sync-BN.

Strategy:
  - Shard batch (4096) across 8 NeuronCores (512 rows each); replicate weights.
  - Activations live in SBUF transposed: [features -> partitions, batch -> free].
    BatchNorm batch stats are then free-axis reductions (VectorE / ACT accum).
  - Matmuls: lhsT = W.T tile (stationary), rhs = xT tile (moving), bf16 in,
    fp32 PSUM accumulation. Weight sign() / transposes / casts done host-side.
  - Sync-BN: per-feature sum / sumsq reduced locally, then one 32KB fp32
    AllReduce per BN layer across the 8 cores.
  - Final Linear flips layout back to [batch -> partitions, classes -> free] by
    using the h3 activation tiles as the stationary operand; log_softmax is a
    free-axis max/exp-accum/ln chain; b3 is folded in via an extra
    ones-row x b3-row contraction tile.
"""

import os
import sys

for _p in ("/opt/trn_rl_repo",):
    if _p not in sys.path and os.path.isdir(_p):
        sys.path.insert(0, _p)

import numpy as np
import ml_dtypes

import concourse.bass as bass
import concourse.mybir as mybir
import concourse.tile as tile
from concourse import bacc
from concourse.bass_utils import run_bass_kernel_spmd

AF = mybir.ActivationFunctionType
ALU = mybir.AluOpType
F32 = mybir.dt.float32
BF16 = mybir.dt.bfloat16
F8 = mybir.dt.float8e4
AX = mybir.AxisListType
PM = mybir.MatmulPerfMode

NP_BF16 = ml_dtypes.bfloat16
NP_F8 = ml_dtypes.float8_e4m3

P = 128
N_CORES = 8
B_TOTAL = 4096
D_IN = 4096
H1, H2, H3 = 4096, 4096, 2048
C = 1000
BN_EPS = 1e-5

KT0, MT0 = D_IN // P, H1 // P  # 32, 32
KT1, MT1 = H1 // P, H2 // P  # 32, 32
KT2, MT2 = H2 // P, H3 // P  # 32, 16
KT3 = H3 // P  # 16 (+1 aug tile for the bias)
MG = 4  # out-feature tiles per PSUM group (4 banks; 2 groups in flight)
KPAIR = 4  # k-tiles per weight-slab DMA (512KB slabs amortize DMA fixed cost)
# fp8 activation offset: h1/h2 are stored as (relu(bn(h)) - OFFC) in e4m3,
# centering the post-ReLU distribution to cut quantization noise ~30%. The
# resulting constant per-feature shift downstream is absorbed by the next
# BatchNorm's mean subtraction. 13/32 is exact in both bf16 and e4m3.
OFFC = 0.40625


def build(b_shard: int, n_cores: int):
    """Build + compile the SPMD program for a per-core batch shard of b_shard."""
    assert b_shard % P == 0
    nb = b_shard // P  # batch tiles for the final layer
    n_batch_global = b_shard * n_cores
    rg = [list(range(n_cores))]

    nc = bacc.Bacc(
        "TRN2", target_bir_lowering=False, debug=False, num_devices=n_cores
    )

    xT = nc.dram_tensor("xT", [D_IN, b_shard], BF16, kind="ExternalInput").ap()
    w0t = nc.dram_tensor("w0t", [D_IN, H1], BF16, kind="ExternalInput").ap()
    w1t = nc.dram_tensor("w1t", [H1, H2], F8, kind="ExternalInput").ap()
    w2t = nc.dram_tensor("w2t", [H2, H3], F8, kind="ExternalInput").ap()
    # W3.T augmented with a b3 row (row H3) + zero padding to a full k-tile.
    w3t = nc.dram_tensor("w3t", [(KT3 + 1) * P, C], BF16, kind="ExternalInput").ap()
    g0p = nc.dram_tensor("g0p", [P, MT0], F32, kind="ExternalInput").ap()
    b0p = nc.dram_tensor("b0p", [P, MT0], F32, kind="ExternalInput").ap()
    g1p = nc.dram_tensor("g1p", [P, MT1], F32, kind="ExternalInput").ap()
    b1p = nc.dram_tensor("b1p", [P, MT1], F32, kind="ExternalInput").ap()
    g2p = nc.dram_tensor("g2p", [P, MT2], F32, kind="ExternalInput").ap()
    b2p = nc.dram_tensor("b2p", [P, MT2], F32, kind="ExternalInput").ap()
    out = nc.dram_tensor("out", [b_shard, C], F32, kind="ExternalOutput").ap()

    with tile.TileContext(nc) as tc:
        with (
            tc.tile_pool(name="big", bufs=1) as big,
            tc.tile_pool(name="wpool", bufs=5) as wpool,
            tc.tile_pool(name="psum", bufs=8, space="PSUM") as psum,
            tc.tile_pool(name="scratch", bufs=4) as scratch,
            tc.tile_pool(name="bn", bufs=8) as bnp,
            tc.tile_pool(name="small", bufs=24) as small,
            tc.tile_pool(name="dram", bufs=1, space="DRAM") as dram,
        ):
            # ---- persistent activation buffers -------------------------------
            # h1/h2 are fp8e4: layers 1/2 run fp8 DoubleRow matmuls (their
            # sign() weights are exactly +-1 in fp8). Raw pre-BN activations
            # stay bf16 in pre0/pre1 until the BN apply quantizes them.
            xT_sb = big.tile([P, KT0, b_shard], BF16, name="xT_sb", tag="xT_sb")
            h1_sb = big.tile([P, MT0, b_shard], F8, name="h1_sb", tag="h1_sb")
            h2_sb = big.tile([P, MT1, b_shard], F8, name="h2_sb", tag="h2_sb")
            h3_sb = big.tile([P, MT2, b_shard], BF16, name="h3_sb", tag="h3_sb")
            # one shared pre-BN scratch: layer N+1's first stats-write lands
            # only after its k-loop consumed ALL of layer N's output, i.e.
            # after every layer-N apply (the last pre readers) completed
            pre0 = big.tile([P, MT0, b_shard], BF16, name="pre0", tag="pre0")
            pre1 = pre0
            ones_t = big.tile([P, b_shard], BF16, name="ones_t", tag="ones_t")

            nc.gpsimd.memset(ones_t[:], 0.0)
            nc.gpsimd.memset(ones_t[:1, :], 1.0)
            # xT loads in 512KB chunks, emitted just-in-time on the sync
            # queue interleaved with the weight-slab stream (one-chunk
            # lookahead) so the first matmuls start within a few us.
            xT_r = xT.rearrange("(ko p) b -> p ko b", p=P)
            XCH = 4  # k-tiles per xT chunk
            n_xch = KT0 // XCH
            xch_emitted = [False] * n_xch

            def xT_loader(kp):
                # sync queue, just-in-time before the slab that needs it
                # (HWDGE ramps faster than SWDGE for these strided chunks)
                want = min((kp + KPAIR - 1) // XCH, n_xch - 1)
                for c in range(want + 1):
                    if not xch_emitted[c]:
                        xch_emitted[c] = True
                        nc.sync.dma_start(
                            xT_sb[:, c * XCH : (c + 1) * XCH, :],
                            xT_r[:, c * XCH : (c + 1) * XCH, :],
                        )

            # BN gamma/beta (host packed to [P, MT])
            gb = {}

            def load_gb(specs):
                for nm, ap_, mt in specs:
                    t = big.tile([P, mt], F32, name=f"{nm}_sb", tag=f"{nm}_sb")
                    nc.gpsimd.dma_start(t[:], ap_)
                    gb[nm] = t

            # only layer 0's params up front; the rest load after layer 0's
            # emission so the startup gpsimd queue stays clear for xT chunks
            load_gb((("g0", g0p, MT0), ("b0", b0p, MT0)))

            # warm the ACT Exp/Ln LUTs now (ACT is idle) so the softmax tail
            # doesn't pay the 1.28us table load on its critical path
            warm = small.tile([P, 1], F32, name="warm", tag="sm")
            nc.gpsimd.memset(warm[:], 1.0)
            nc.scalar.activation(warm[:], warm[:], AF.Exp)
            nc.scalar.activation(warm[:], warm[:], AF.Ln)

            def mlp_layer(
                lidx, in_sb, kt, mt, w_dram, g_sb, b_sb, out_sb, pre_sb,
                a_groups=None, cover_k=None, in_loader=None,
                fp8=False, out_off=None,
            ):
                """out_sb <- relu(bn(in_sb.T @ W.T)), transposed layout.

                Pipelined sync-BN: per-feature stats are all-reduced in two
                chunks (chunk-a = first a_groups PSUM groups); AR-a overlaps
                this layer's later matmuls and AR-b overlaps the NEXT layer's
                matmuls, whose k-loops are split at cover_k so input tiles
                normalized by the previous chunk-a are consumed first.

                fp8: in_sb + weights are fp8e4; matmuls run DoubleRow (2
                k-tiles per instruction, 2x PE throughput).
                out_off: offset-coded fp8 output — store relu(bn(h)) - C.
                The induced per-feature shift C*colsum(W) in the NEXT layer's
                pre-activations is constant over the batch, so that layer's
                BatchNorm mean-subtraction absorbs it exactly; no correction
                term is needed anywhere.
                """
                ngroups = mt // MG
                half_g = a_groups if a_groups is not None else ngroups // 2
                a_m = half_g * MG  # chunk-a feature tiles
                b_m = mt - a_m
                kh = cover_k if cover_k is not None else kt // 2
                inv_n = 1.0 / float(n_batch_global)
                chunk_m = [a_m, b_m]

                stats = [
                    big.tile(
                        [P, 2 * chunk_m[c]], F32, name=f"stats{lidx}_{c}",
                        tag=f"stats{lidx}_{c}",
                    )
                    for c in range(2)
                ]
                gstats = [
                    big.tile(
                        [P, 2 * chunk_m[c]], F32, name=f"gstats{lidx}_{c}",
                        tag=f"gstats{lidx}_{c}",
                    )
                    for c in range(2)
                ]
                arin = [
                    dram.tile(
                        [P, 2 * chunk_m[c]], F32, name=f"arin{lidx}_{c}",
                        tag=f"arin{lidx}_{c}",
                    )
                    for c in range(2)
                ]
                arout = [
                    dram.tile(
                        [P, 2 * chunk_m[c]], F32, name=f"arout{lidx}_{c}",
                        tag=f"arout{lidx}_{c}",
                    )
                    for c in range(2)
                ]
                s_sb = [
                    big.tile(
                        [P, chunk_m[c]], F32, name=f"s{lidx}_{c}", tag=f"s{lidx}_{c}"
                    )
                    for c in range(2)
                ]
                t_sb = [
                    big.tile(
                        [P, chunk_m[c]], F32, name=f"t{lidx}_{c}", tag=f"t{lidx}_{c}"
                    )
                    for c in range(2)
                ]

                ps_tiles = {}

                def emit_mms(g, k_lo, k_hi):
                    if g not in ps_tiles:
                        ps_tiles[g] = [
                            psum.tile(
                                [P, b_shard], F32, name=f"ps{lidx}_{g}_{j}", tag="ps"
                            )
                            for j in range(MG)
                        ]
                    ps = ps_tiles[g]
                    for kp in range(k_lo, k_hi, KPAIR):
                        if in_loader is not None:
                            in_loader(kp)
                        slab = wpool.tile(
                            [P, KPAIR, MG * P], F8 if fp8 else BF16,
                            name=f"w{lidx}_{g}_{kp}", tag="wslab",
                        )
                        nc.sync.dma_start(
                            slab[:],
                            w_dram[
                                kp * P : (kp + KPAIR) * P,
                                g * MG * P : (g + 1) * MG * P,
                            ].rearrange("(kk p) c -> p kk c", p=P),
                        )
                        if fp8:
                            for kk in range(0, KPAIR, 2):
                                k = kp + kk
                                for j in range(MG):
                                    nc.tensor.matmul(
                                        ps[j][:],
                                        slab[:, kk : kk + 2, j * P : (j + 1) * P],
                                        in_sb[:, k : k + 2, :],
                                        start=(k == 0),
                                        stop=(k == kt - 2),
                                        perf_mode=PM.DoubleRow,
                                    )
                        else:
                            for kk in range(KPAIR):
                                k = kp + kk
                                for j in range(MG):
                                    nc.tensor.matmul(
                                        ps[j][:],
                                        slab[:, kk, j * P : (j + 1) * P],
                                        in_sb[:, k, :],
                                        start=(k == 0),
                                        stop=(k == kt - 1),
                                    )

                def emit_stats(g):
                    c = 0 if g < half_g else 1
                    st = stats[c]
                    cm = chunk_m[c]
                    for j in range(MG):
                        m = g * MG + j
                        ml = m - c * a_m
                        # DVE: PSUM -> bf16 pre-BN copy, fused with the
                        # per-feature sum via accum_out (one PSUM read)
                        nc.vector.tensor_scalar(
                            pre_sb[:, m, :],
                            ps_tiles[g][j][:],
                            1.0,
                            None,
                            ALU.mult,
                            ALU.add,  # accum reduce op: sum over the batch
                            accum_out=st[:, ml : ml + 1],
                        )
                        sq = scratch.tile(
                            [P, b_shard], F32, name=f"sq{lidx}_{m}", tag="sq"
                        )
                        # HW: only one PSUM read per DVE inst, so square on ACT
                        nc.scalar.activation(
                            sq[:],
                            ps_tiles[g][j][:],
                            AF.Square,
                            accum_out=st[:, cm + ml : cm + ml + 1],
                        )

                def emit_ar(c):
                    nc.gpsimd.dma_start(arin[c][:], stats[c][:])
                    nc.gpsimd.collective_compute(
                        "AllReduce",
                        ALU.add,
                        replica_groups=rg,
                        ins=[arin[c].opt()],
                        outs=[arout[c].opt()],
                    )
                    nc.gpsimd.dma_start(gstats[c][:], arout[c][:])

                def emit_apply(c):
                    # s = g * rsqrt(var+eps); t = beta - mean*s, then in-place
                    # relu(h*s + t) for this chunk's feature tiles.
                    gs = gstats[c]
                    cm = chunk_m[c]
                    m0 = c * a_m
                    mex = bnp.tile([P, 2 * cm], F32, name=f"mex{lidx}_{c}", tag="bn2")
                    m2 = bnp.tile([P, cm], F32, name=f"m2{lidx}_{c}", tag="bn")
                    var = bnp.tile([P, cm], F32, name=f"var{lidx}_{c}", tag="bn")
                    inv = bnp.tile([P, cm], F32, name=f"inv{lidx}_{c}", tag="bn")
                    rstd = bnp.tile([P, cm], F32, name=f"rstd{lidx}_{c}", tag="bn")
                    tmp = bnp.tile([P, cm], F32, name=f"tmp{lidx}_{c}", tag="bn")
                    nc.scalar.activation(mex[:], gs[:], AF.Copy, scale=inv_n)
                    mean = mex[:, :cm]
                    ex2 = mex[:, cm:]
                    nc.vector.tensor_mul(m2[:], mean[:], mean[:])
                    nc.vector.tensor_sub(var[:], ex2[:], m2[:])
                    nc.vector.tensor_scalar_add(var[:], var[:], BN_EPS)
                    nc.vector.reciprocal(inv[:], var[:])
                    nc.scalar.activation(rstd[:], inv[:], AF.Sqrt)
                    nc.vector.tensor_mul(
                        s_sb[c][:], rstd[:], g_sb[:, m0 : m0 + cm]
                    )
                    nc.vector.tensor_mul(tmp[:], mean[:], s_sb[c][:])
                    nc.vector.tensor_sub(
                        t_sb[c][:], b_sb[:, m0 : m0 + cm], tmp[:]
                    )
                    for ml in range(cm):
                        m = m0 + ml
                        if out_off is None:
                            nc.scalar.activation(
                                out_sb[:, m, :],
                                pre_sb[:, m, :],
                                AF.Relu,
                                bias=t_sb[c][:, ml : ml + 1],
                                scale=s_sb[c][:, ml : ml + 1],
                            )
                        else:
                            # relu in-place on the bf16 pre tile (ACT), then
                            # offset-subtract + fp8 cast on DVE
                            nc.scalar.activation(
                                pre_sb[:, m, :],
                                pre_sb[:, m, :],
                                AF.Relu,
                                bias=t_sb[c][:, ml : ml + 1],
                                scale=s_sb[c][:, ml : ml + 1],
                            )
                            nc.vector.tensor_scalar(
                                out_sb[:, m, :],
                                pre_sb[:, m, :],
                                out_off,
                                None,
                                ALU.subtract,
                            )

                # groups 0,1: k-loop split so the first half only needs the
                # previous layer's chunk-a (covers that layer's AR-b latency).
                # For the input layer, interleave in 4-k sub-blocks so the PE
                # can start as soon as the first 512KB input chunk lands.
                if in_loader is not None:
                    for k0 in range(0, kh, 4):
                        emit_mms(0, k0, k0 + 4)
                        emit_mms(1, k0, k0 + 4)
                else:
                    emit_mms(0, 0, kh)
                    emit_mms(1, 0, kh)
                emit_mms(0, kh, kt)
                emit_stats(0)
                emit_mms(1, kh, kt)
                emit_stats(1)
                for g in range(2, half_g):
                    emit_mms(g, 0, kt)
                    emit_stats(g)
                emit_ar(0)  # chunk-a stats AR overlaps chunk-b matmuls
                apply_a_at = half_g + 1 if (ngroups - half_g) > 2 else None
                apply_a_done = False
                for g in range(half_g, ngroups):
                    if apply_a_at is None and not apply_a_done:
                        # small chunk-b: wedge the chunk-a BN apply between
                        # this group's two half-k blocks so it runs as soon
                        # as AR-a lands, not after the whole layer
                        emit_mms(g, 0, kh)
                        emit_apply(0)
                        apply_a_done = True
                        emit_mms(g, kh, kt)
                        emit_stats(g)
                        continue
                    emit_mms(g, 0, kt)
                    emit_stats(g)
                    if apply_a_at is not None and g == apply_a_at:
                        emit_apply(0)
                        apply_a_done = True
                if not apply_a_done:
                    emit_apply(0)
                emit_ar(1)
                emit_apply(1)

            # chunk-a = 6 of 8 groups: the trailing AR covers ~50us of the
            # next layer's split matmuls (ARs vary 10-37us run to run)
            mlp_layer(
                0, xT_sb, KT0, MT0, w0t, gb["g0"], gb["b0"], h1_sb, pre0,
                a_groups=6, in_loader=xT_loader, out_off=OFFC,
            )
            load_gb(
                (
                    ("g1", g1p, MT1),
                    ("b1", b1p, MT1),
                    ("g2", g2p, MT2),
                    ("b2", b2p, MT2),
                )
            )
            mlp_layer(
                1, h1_sb, KT1, MT1, w1t, gb["g1"], gb["b1"], h2_sb, pre1,
                a_groups=6, cover_k=24, fp8=True, out_off=OFFC,
            )

            # preload ALL final-layer weight slabs now — the DMAs run during
            # layer 2's compute and layer 3 then never waits on weight loads
            # gpsimd (SWDGE) queue: runs during layer 2 without delaying the
            # sync-queue weight-slab stream
            w3_sb = big.tile([P, KT3 + 1, C], BF16, name="w3_sb", tag="w3_sb")
            nc.gpsimd.dma_start(
                w3_sb[:], w3t.rearrange("(ko p) c -> p ko c", p=P)
            )

            # asymmetric chunks (3+1 groups): the tiny chunk-b AR is covered
            # by layer 3's first 12 k-tiles
            mlp_layer(
                2, h2_sb, KT2, MT2, w2t, gb["g2"], gb["b2"], h3_sb, pre0,
                a_groups=3, cover_k=24, fp8=True,
            )

            # ---- final Linear + log_softmax ---------------------------------
            # lhsT = h3 tile slice (stationary), rhs = preloaded W3.T slab
            # (moving). Output flips to [batch -> partitions, classes -> free].
            # k-loop split: first 8 k-tiles (layer 2's chunk-a) for every
            # batch tile first, covering layer 2's second stats-AR.
            half = (C + 1) // 2  # 500
            ka = 12  # layer 2's chunk-a feature tiles
            ps3 = [
                [
                    psum.tile([P, 512], F32, name=f"ps3_{b}_{h}", tag="ps")
                    for h in range(2)
                ]
                for b in range(nb)
            ]

            def l3_mms(b, ks, stop_k, start_k=None):
                for k in ks:
                    lhsT = (
                        h3_sb[:, k, b * P : (b + 1) * P]
                        if k < KT3
                        else ones_t[:, b * P : (b + 1) * P]
                    )
                    for h in range(2):
                        nc.tensor.matmul(
                            ps3[b][h][:, : half],
                            lhsT,
                            w3_sb[:, k, h * half : (h + 1) * half],
                            start=(k == start_k),
                            stop=(k == stop_k),
                        )

            for b in range(nb):
                # bias (ones) k-tile first — it's ungated by layer 2's BN
                # applies, so it runs while layer 2's stats-ARs are in flight
                l3_mms(b, [KT3] + list(range(ka)), None, start_k=KT3)

            # log_softmax, stage-batched across batch tiles so the ACT LUT
            # (Exp / Ln) is loaded once per stage instead of per tile
            nmax = [None] * nb
            s0 = [None] * nb
            s1 = [None] * nb
            lse = [None] * nb
            shift = [None] * nb
            for b in range(nb):
                l3_mms(b, range(ka, KT3), KT3 - 1)
                p0 = ps3[b][0][:, :half]
                p1 = ps3[b][1][:, :half]
                m0 = small.tile([P, 1], F32, name=f"m0_{b}", tag="sm")
                m1 = small.tile([P, 1], F32, name=f"m1_{b}", tag="sm")
                nmax[b] = small.tile([P, 1], F32, name=f"nmax_{b}", tag="sm")
                nc.vector.tensor_reduce(m0[:], p0, axis=AX.X, op=ALU.max)
                nc.vector.tensor_reduce(m1[:], p1, axis=AX.X, op=ALU.max)
                nc.vector.tensor_max(m0[:], m0[:], m1[:])
                nc.vector.tensor_scalar_mul(nmax[b][:], m0[:], -1.0)
            for b in range(nb):
                s0[b] = small.tile([P, 1], F32, name=f"s0_{b}", tag="sm")
                s1[b] = small.tile([P, 1], F32, name=f"s1_{b}", tag="sm")
                e0 = scratch.tile([P, 512], F32, name=f"e0_{b}", tag="sq")
                e1 = scratch.tile([P, 512], F32, name=f"e1_{b}", tag="sq")
                nc.scalar.activation(
                    e0[:, :half], ps3[b][0][:, :half], AF.Exp,
                    bias=nmax[b][:], scale=1.0, accum_out=s0[b][:],
                )
                nc.scalar.activation(
                    e1[:, :half], ps3[b][1][:, :half], AF.Exp,
                    bias=nmax[b][:], scale=1.0, accum_out=s1[b][:],
                )
            for b in range(nb):
                ssum = small.tile([P, 1], F32, name=f"ssum_{b}", tag="sm")
                lse[b] = small.tile([P, 1], F32, name=f"lse_{b}", tag="sm")
                nc.vector.tensor_add(ssum[:], s0[b][:], s1[b][:])
                nc.scalar.activation(lse[b][:], ssum[:], AF.Ln)
            for b in range(nb):
                shift[b] = small.tile([P, 1], F32, name=f"shift_{b}", tag="sm")
                nc.vector.tensor_sub(shift[b][:], nmax[b][:], lse[b][:])
            for b in range(nb):
                # writeback split across ACT and DVE so the two halves of
                # each tile shift in parallel
                o0 = scratch.tile([P, 512], F32, name=f"o0_{b}", tag="sq")
                o1 = scratch.tile([P, 512], F32, name=f"o1_{b}", tag="sq")
                nc.scalar.activation(
                    o0[:, :half], ps3[b][0][:, :half], AF.Identity,
                    bias=shift[b][:], scale=1.0,
                )
                nc.vector.tensor_scalar_add(
                    o1[:, :half], ps3[b][1][:, :half], shift[b][:]
                )
                # halves on different DMA queues so the 8 stores drain in
                # parallel instead of serializing on sync
                nc.sync.dma_start(out[b * P : (b + 1) * P, :half], o0[:, :half])
                nc.gpsimd.dma_start(out[b * P : (b + 1) * P, half:C], o1[:, :half])

    nc.compile()
    return nc


def prep_inputs(inputs, b_shard: int, n_cores: int):
    """Host-side prep: shard x, transpose/cast weights, pack BN params."""
    x = np.ascontiguousarray(inputs["x"], dtype=np.float32)

    def bf(a):
        return np.ascontiguousarray(a).astype(NP_BF16)

    def sign_f32(w):
        return np.where(w >= 0, np.float32(1.0), np.float32(-1.0))

    def f8(a):
        return np.ascontiguousarray(a).astype(NP_F8)

    w0t = bf(inputs["W0"].astype(np.float32).T)
    w1t = f8(sign_f32(np.asarray(inputs["Wb1"], dtype=np.float32)).T)
    w2t = f8(sign_f32(np.asarray(inputs["Wb2"], dtype=np.float32)).T)
    w3t_aug = np.zeros(((KT3 + 1) * P, C), dtype=np.float32)
    w3t_aug[:H3] = inputs["W3"].astype(np.float32).T
    w3t_aug[H3] = inputs["b3"].astype(np.float32)
    w3t_aug = bf(w3t_aug)

    def pack(v, mt):
        return np.ascontiguousarray(
            np.asarray(v, dtype=np.float32).reshape(mt, P).T
        )

    shared = {
        "w0t": w0t,
        "w1t": w1t,
        "w2t": w2t,
        "w3t": w3t_aug,
        "g0p": pack(inputs["g0"], MT0),
        "b0p": pack(inputs["beta0"], MT0),
        "g1p": pack(inputs["g1"], MT1),
        "b1p": pack(inputs["beta1"], MT1),
        "g2p": pack(inputs["g2"], MT2),
        "b2p": pack(inputs["beta2"], MT2),
    }
    in_maps = []
    for i in range(n_cores):
        xs = x[i * b_shard : (i + 1) * b_shard]  # [b_shard, D_IN]
        m = dict(shared)
        m["xT"] = bf(xs.T)  # [D_IN, b_shard]
        in_maps.append(m)
    return in_maps


_CACHE = {}


def _get_compiled(b_shard: int, n_cores: int):
    key = (b_shard, n_cores)
    if key not in _CACHE:
        _CACHE[key] = build(b_shard, n_cores)
    return _CACHE[key]


def kernel(**inputs) -> np.ndarray:
    b_shard = B_TOTAL // N_CORES
    nc = _get_compiled(b_shard, N_CORES)
    in_maps = prep_inputs(inputs, b_shard, N_CORES)
    last_err = None
    for _attempt in range(3):
        try:
            res = run_bass_kernel_spmd(nc, in_maps, core_ids=list(range(N_CORES)))
            break
        except Exception as e:  # transient NRT device flakes recover on retry
            last_err = e
    else:
        raise last_err
    out = np.concatenate([r["out"] for r in res.results], axis=0)
    return out.astype(np.float32)


if __name__ == "__main__":
    data = np.load("/tmp/ref_data.npz")
    inputs = {k: data[k] for k in data.files if k != "expected"}
    expected = data["expected"]
    actual = kernel(**inputs)
    err = np.abs(actual - expected)
    print("max abs err:", err.max())
    print("absmax-rel:", err.max() / np.abs(expected).max())

